# revision 2
# baseline (speedup 1.0000x reference)
"""Trainium2 Bass kernel for CustomGraphConvLayer (GNN message passing).

out = relu(x @ W_self.T + b_self + [count>0]((segmean x[dst] by src) @ W_neighbor.T + b_neighbor))

Strategy (8 NeuronCores, SPMD):
  - 1D node partition: core k owns nodes [k*NPC, (k+1)*NPC).
  - Edges routed to the core owning their src node; x replicated per core so
    all dst gathers are core-local HBM reads (no collectives).
  - Per core: dma_gather 512B rows of x[dst] in 128-edge chunks; segment-sum
    via one-hot matmuls on the tensor engine into per-128-node-block PSUM;
    scale by 1/count, transpose, then fused self+neighbor matmul + bias +
    relu, DMA out.
  - SPMD: per-(block,half) chunk counts are padded to the max across cores so
    a single program serves all 8 cores; pad edges gather row 0 with a
    sentinel local-segment id (-1) whose one-hot column is all zero.
"""

import sys

for _p in ("/opt/trn_rl_repo", "/root/.axon_site/_ro/trn_rl_repo"):
    if _p not in sys.path:
        sys.path.append(_p)

import numpy as np

import concourse.bass as bass
import concourse.tile as tile
from concourse import bacc, mybir
from concourse.bass_utils import run_bass_kernel_spmd

N_NODES = 50000
N_EDGES = 800000
D = 128
N_CORES = 8
NPC = N_NODES // N_CORES          # 6250 nodes per core
NBLK = (NPC + 127) // 128         # 49 blocks of 128 nodes
NPAD = NBLK * 128                 # 6272
HALF = N_NODES // 2               # 25000 (int16 gather-index window size)

F32 = mybir.dt.float32
I16 = mybir.dt.int16


def _preprocess(x, edge_index, W_self, b_self, W_neighbor, b_neighbor):
    """Route edges to cores, build per-core gather/segment metadata.

    Returns (in_maps, C, CO, TOT):
      in_maps: list of 8 dicts of named numpy inputs
      C[b][h]: chunk count for (block b, half h), uniform across cores
      CO[b][h]: chunk offset of group (b,h) in the stream
      TOT: total chunks
    """
    src = np.asarray(edge_index[0], dtype=np.int64)
    dst = np.asarray(edge_index[1], dtype=np.int64)
    x = np.asarray(x, dtype=np.float32)

    counts = np.bincount(src, minlength=N_NODES).astype(np.int64)

    core = src // NPC
    lid = src - core * NPC
    blk = lid >> 7
    lseg = lid & 127
    half = (dst >= HALF).astype(np.int64)

    # group key: (core, blk, half)
    key = (core * NBLK + blk) * 2 + half
    ngroups = N_CORES * NBLK * 2
    gcount = np.bincount(key, minlength=ngroups)                # edges per group
    # chunk counts per (blk, half): max over cores, at least enough for edges
    C = np.ceil(gcount.reshape(N_CORES, NBLK, 2) / 128.0).astype(np.int64).max(axis=0)
    TOT = int(C.sum())
    CO = np.zeros((NBLK, 2), dtype=np.int64)
    CO.flat[1:] = np.cumsum(C.flat)[:-1]

    # stable order groups edges by (core, blk, half)
    order = np.argsort(key, kind="stable")
    skey = key[order]
    gstart = np.zeros(ngroups, dtype=np.int64)
    gstart[1:] = np.cumsum(gcount)[:-1]
    rank = np.arange(N_EDGES, dtype=np.int64) - gstart[skey]    # rank within group

    # slot within the per-core stream (same layout for every core)
    ebase = (CO * 128).reshape(-1)                              # [NBLK*2]
    bh = skey % (NBLK * 2)
    slot = ebase[bh] + rank
    score = skey // (NBLK * 2)                                  # owning core, sorted asc

    sdst = dst[order]
    shalf = half[order]
    slseg = lseg[order]

    nslots = TOT * 128
    W_self_T = np.ascontiguousarray(np.asarray(W_self, np.float32).T)
    W_nb_T = np.ascontiguousarray(np.asarray(W_neighbor, np.float32).T)
    bs_row = np.asarray(b_self, np.float32).reshape(1, D).copy()
    bn_row = np.asarray(b_neighbor, np.float32).reshape(1, D).copy()
    cmpidx = np.tile(np.arange(128, dtype=np.float32), (128, 1))
    ident = np.eye(128, dtype=np.float32)

    in_maps = []
    for k in range(N_CORES):
        m = score == k
        idx_arr = np.zeros(nslots, dtype=np.int16)
        lseg_arr = np.full(nslots, -1.0, dtype=np.float32)
        ks = slot[m]
        idx_arr[ks] = (sdst[m] - shalf[m] * HALF).astype(np.int16)
        lseg_arr[ks] = slseg[m].astype(np.float32)

        # gather-index layout: element j of a call -> [j%16, j//16], replicated
        # over the 8 groups of 16 partitions
        idx16 = np.ascontiguousarray(
            np.tile(idx_arr.reshape(-1, 16).T, (8, 1))
        )                                                        # [128, TOT*8]
        lsegT = np.ascontiguousarray(lseg_arr.reshape(TOT, 128).T)  # [128, TOT]

        c_own = counts[k * NPC : (k + 1) * NPC]
        c_pad = np.concatenate([c_own, np.zeros(NPAD - NPC, np.int64)])
        inv = (1.0 / np.maximum(c_pad, 1)).astype(np.float32)
        inv_cnt = np.ascontiguousarray(inv.reshape(NBLK, 128).T)    # [128, NBLK]
        maskrow = (c_pad > 0).astype(np.float32).reshape(1, NPAD).copy()

        xo = x[k * NPC : (k + 1) * NPC]
        x_ownT = np.zeros((D, NPAD), dtype=np.float32)
        x_ownT[:, :NPC] = xo.T

        in_maps.append(
            {
                "x_full": x,
                "x_ownT": x_ownT,
                "idx16": idx16,
                "lsegT": lsegT,
                "inv_cnt": inv_cnt,
                "maskrow": maskrow,
                "W_self_T": W_self_T,
                "W_nb_T": W_nb_T,
                "bs_row": bs_row,
                "bn_row": bn_row,
                "cmpidx": cmpidx,
                "ident": ident,
            }
        )
    return in_maps, C, CO, TOT


def _build(C, CO, TOT):
    nc = bacc.Bacc("TRN2", target_bir_lowering=True)

    x_full = nc.dram_tensor("x_full", [N_NODES, D], F32, kind="ExternalInput")
    x_ownT = nc.dram_tensor("x_ownT", [D, NPAD], F32, kind="ExternalInput")
    idx16 = nc.dram_tensor("idx16", [128, TOT * 8], I16, kind="ExternalInput")
    lsegT = nc.dram_tensor("lsegT", [128, TOT], F32, kind="ExternalInput")
    inv_cnt = nc.dram_tensor("inv_cnt", [128, NBLK], F32, kind="ExternalInput")
    maskrow = nc.dram_tensor("maskrow", [1, NPAD], F32, kind="ExternalInput")
    W_self_T = nc.dram_tensor("W_self_T", [D, D], F32, kind="ExternalInput")
    W_nb_T = nc.dram_tensor("W_nb_T", [D, D], F32, kind="ExternalInput")
    bs_row = nc.dram_tensor("bs_row", [1, D], F32, kind="ExternalInput")
    bn_row = nc.dram_tensor("bn_row", [1, D], F32, kind="ExternalInput")
    cmpidx = nc.dram_tensor("cmpidx", [128, 128], F32, kind="ExternalInput")
    ident = nc.dram_tensor("ident", [128, 128], F32, kind="ExternalInput")
    out = nc.dram_tensor("out", [NPC, D], F32, kind="ExternalOutput")

    with tile.TileContext(nc) as tc:
        with (
            tc.tile_pool(name="consts", bufs=1) as consts,
            tc.tile_pool(name="gp", bufs=4) as gp,
            tc.tile_pool(name="ohp", bufs=4) as ohp,
            tc.tile_pool(name="mp", bufs=3) as mp,
            tc.tile_pool(name="pseg", bufs=2, space="PSUM") as pseg,
            tc.tile_pool(name="ptp", bufs=2, space="PSUM") as ptp,
            tc.tile_pool(name="pop", bufs=2, space="PSUM") as pop,
        ):
            idx_sb = consts.tile([128, TOT * 8], I16)
            nc.sync.dma_start(out=idx_sb, in_=idx16[:, :])
            lseg_sb = consts.tile([128, TOT], F32)
            nc.sync.dma_start(out=lseg_sb, in_=lsegT[:, :])
            xoT_sb = consts.tile([128, NPAD], F32)
            nc.sync.dma_start(out=xoT_sb, in_=x_ownT[:, :])
            inv_sb = consts.tile([128, NBLK], F32)
            nc.sync.dma_start(out=inv_sb, in_=inv_cnt[:, :])
            mask_sb = consts.tile([1, NPAD], F32)
            nc.sync.dma_start(out=mask_sb, in_=maskrow[:, :])
            wsT_sb = consts.tile([128, 128], F32)
            nc.sync.dma_start(out=wsT_sb, in_=W_self_T[:, :])
            wnT_sb = consts.tile([128, 128], F32)
            nc.sync.dma_start(out=wnT_sb, in_=W_nb_T[:, :])
            bs_sb = consts.tile([1, 128], F32)
            nc.sync.dma_start(out=bs_sb, in_=bs_row[:, :])
            bn_sb = consts.tile([1, 128], F32)
            nc.sync.dma_start(out=bn_sb, in_=bn_row[:, :])
            cmp_sb = consts.tile([128, 128], F32)
            nc.sync.dma_start(out=cmp_sb, in_=cmpidx[:, :])
            id_sb = consts.tile([128, 128], F32)
            nc.sync.dma_start(out=id_sb, in_=ident[:, :])
            ones_sb = consts.tile([1, 128], F32)
            nc.vector.memset(ones_sb, 1.0)

            for b in range(NBLK):
                nmm = int(C[b][0] + C[b][1])
                mmi = 0
                if nmm > 0:
                    seg = pseg.tile([128, 128], F32, tag="seg")
                for h in (0, 1):
                    cb = int(C[b][h])
                    if cb == 0:
                        continue
                    co = int(CO[b][h])
                    g = gp.tile([128, cb, 128], F32, tag="g")
                    nc.gpsimd.dma_gather(
                        out_ap=g[:, :, :],
                        in_ap=x_full[h * HALF : (h + 1) * HALF, :],
                        idxs_ap=idx_sb[:, co * 8 : (co + cb) * 8],
                        num_idxs=cb * 128,
                        num_idxs_reg=cb * 128,
                        elem_size=D,
                        single_packet=False,
                    )
                    for c in range(cb):
                        oh = ohp.tile([128, 128], F32, tag="oh")
                        nc.vector.tensor_scalar(
                            out=oh,
                            in0=cmp_sb,
                            scalar1=lseg_sb[:, co + c : co + c + 1],
                            scalar2=None,
                            op0=mybir.AluOpType.is_equal,
                        )
                        nc.tensor.matmul(
                            seg,
                            lhsT=oh,
                            rhs=g[:, c, :],
                            start=(mmi == 0),
                            stop=(mmi == nmm - 1),
                        )
                        mmi += 1

                if nmm > 0:
                    mean = mp.tile([128, 128], F32, tag="mean")
                    nc.vector.tensor_scalar_mul(mean, seg[:, :], inv_sb[:, b : b + 1])
                    pt = ptp.tile([128, 128], F32, tag="pt")
                    nc.tensor.transpose(pt, mean, id_sb)
                    meanT = mp.tile([128, 128], F32, tag="meanT")
                    nc.vector.tensor_copy(meanT, pt[:, :])

                po = pop.tile([128, 128], F32, tag="po")
                nc.tensor.matmul(
                    po, lhsT=xoT_sb[:, b * 128 : (b + 1) * 128], rhs=wsT_sb,
                    start=True, stop=False,
                )
                nc.tensor.matmul(po, lhsT=ones_sb, rhs=bs_sb, start=False, stop=False)
                if nmm > 0:
                    nc.tensor.matmul(po, lhsT=meanT, rhs=wnT_sb, start=False, stop=False)
                nc.tensor.matmul(
                    po, lhsT=mask_sb[:, b * 128 : (b + 1) * 128], rhs=bn_sb,
                    start=False, stop=True,
                )

                ob = mp.tile([128, 128], F32, tag="ob")
                nc.scalar.activation(ob, po[:, :], mybir.ActivationFunctionType.Relu)
                nrows = min(128, NPC - b * 128)
                nc.sync.dma_start(
                    out=out[b * 128 : b * 128 + nrows, :], in_=ob[:nrows, :]
                )

    nc.finalize()
    return nc


def kernel(x, edge_index, W_self, b_self, W_neighbor, b_neighbor):
    in_maps, C, CO, TOT = _preprocess(
        x, edge_index, W_self, b_self, W_neighbor, b_neighbor
    )
    nc = _build(C, CO, TOT)
    res = run_bass_kernel_spmd(nc, in_maps, core_ids=list(range(N_CORES)))
    return np.concatenate([res.results[k]["out"] for k in range(N_CORES)], axis=0)


# exposed for test.py so the perf harness can reuse the prepared pieces
def _prepare(x, edge_index, W_self, b_self, W_neighbor, b_neighbor):
    in_maps, C, CO, TOT = _preprocess(
        x, edge_index, W_self, b_self, W_neighbor, b_neighbor
    )
    nc = _build(C, CO, TOT)
    return nc, in_maps


# revision 4
# speedup vs baseline: 1.1635x; 1.1635x over previous
"""Trainium2 Bass kernel for CustomGraphConvLayer (GNN message passing).

out = relu(x @ W_self.T + b_self + [count>0]((segmean x[dst] by src) @ W_neighbor.T + b_neighbor))

Strategy (8 NeuronCores, SPMD):
  - 1D node partition with degree-balanced striping: nodes sorted by degree,
    dealt round-robin to cores so per-128-node-block edge counts match across
    cores (minimizes SPMD chunk padding). Output rows un-permuted on host.
  - Edges routed to the core owning their src node; the bf16 gather table
    (x cast to bf16) is replicated per core so all dst gathers are local.
  - Per core: dma_gather 256B bf16 rows of x[dst] in 128-edge chunks
    (SWDGE is ~8.3ns/idx of GpSimd time - the critical path); segment-sum via
    bf16 one-hot matmuls on the tensor engine into per-block PSUM (f32
    accumulate); scale by 1/count, PE-transpose, then fused f32
    self+neighbor matmul with rank-1 bias terms, ReLU, DMA out.
  - SPMD: per-(block,half) chunk counts are padded to the max across cores so
    a single program serves all 8 cores; pad edges gather row 0 with a
    sentinel local-segment id (-1) whose one-hot column is all zero.
"""

import sys

for _p in ("/opt/trn_rl_repo", "/root/.axon_site/_ro/trn_rl_repo"):
    if _p not in sys.path:
        sys.path.append(_p)

import ml_dtypes
import numpy as np

import concourse.bass as bass
import concourse.tile as tile
from concourse import bacc, mybir
from concourse.bass_utils import run_bass_kernel_spmd

N_NODES = 50000
N_EDGES = 800000
D = 128
N_CORES = 8
NPC = N_NODES // N_CORES          # 6250 nodes per core
NBLK = (NPC + 127) // 128         # 49 blocks of 128 nodes
NPAD = NBLK * 128                 # 6272
HALF = N_NODES // 2               # 25000 (int16 gather-index window size)

F32 = mybir.dt.float32
BF16 = mybir.dt.bfloat16
I16 = mybir.dt.int16
BF = ml_dtypes.bfloat16


def _preprocess(x, edge_index, W_self, b_self, W_neighbor, b_neighbor):
    """Route edges to cores (degree-balanced), build per-core metadata.

    Returns (in_maps, C, CO, TOT, nodeof):
      in_maps: list of 8 dicts of named numpy inputs
      C[b][h]: chunk count for (block b, half h), uniform across cores
      CO[b][h]: chunk offset of group (b,h) in the stream
      TOT: total chunks
      nodeof[k][p]: original node id at (core k, position p) for output gather
    """
    src = np.asarray(edge_index[0], dtype=np.int64)
    dst = np.asarray(edge_index[1], dtype=np.int64)
    x = np.asarray(x, dtype=np.float32)

    counts = np.bincount(src, minlength=N_NODES).astype(np.int64)

    # degree-balanced striping: rank nodes by degree, node of rank r goes to
    # core r%8 at position r//8 -> per-block edge counts align across cores
    rank_order = np.argsort(-counts, kind="stable")      # node ids by degree desc
    core_of = np.empty(N_NODES, dtype=np.int64)
    pos_of = np.empty(N_NODES, dtype=np.int64)
    r = np.arange(N_NODES)
    core_of[rank_order] = r % N_CORES
    pos_of[rank_order] = r // N_CORES
    nodeof = np.empty((N_CORES, NPC), dtype=np.int64)
    nodeof[core_of[rank_order], pos_of[rank_order]] = rank_order

    core = core_of[src]
    lid = pos_of[src]
    blk = lid >> 7
    lseg = lid & 127
    half = (dst >= HALF).astype(np.int64)

    key = (core * NBLK + blk) * 2 + half
    ngroups = N_CORES * NBLK * 2
    gcount = np.bincount(key, minlength=ngroups)
    C = np.ceil(gcount.reshape(N_CORES, NBLK, 2) / 128.0).astype(np.int64).max(axis=0)
    TOT = int(C.sum())
    CO = np.zeros((NBLK, 2), dtype=np.int64)
    CO.flat[1:] = np.cumsum(C.flat)[:-1]

    order = np.argsort(key, kind="stable")
    skey = key[order]
    gstart = np.zeros(ngroups, dtype=np.int64)
    gstart[1:] = np.cumsum(gcount)[:-1]
    rank = np.arange(N_EDGES, dtype=np.int64) - gstart[skey]

    ebase = (CO * 128).reshape(-1)
    bh = skey % (NBLK * 2)
    slot = ebase[bh] + rank
    score = skey // (NBLK * 2)

    sdst = dst[order]
    shalf = half[order]
    slseg = lseg[order]

    nslots = TOT * 128
    x_bf = x.astype(BF)                                   # gather table, bf16
    W_self_T = np.ascontiguousarray(np.asarray(W_self, np.float32).T)
    W_nb_T = np.ascontiguousarray(np.asarray(W_neighbor, np.float32).T)
    bs_row = np.asarray(b_self, np.float32).reshape(1, D).copy()
    bn_row = np.asarray(b_neighbor, np.float32).reshape(1, D).copy()
    cmpidx = np.tile(np.arange(128, dtype=np.float32), (128, 1)).astype(BF)
    ident = np.eye(128, dtype=np.float32)

    in_maps = []
    for k in range(N_CORES):
        m = score == k
        idx_arr = np.zeros(nslots, dtype=np.int16)
        lseg_arr = np.full(nslots, -1.0, dtype=np.float32)
        ks = slot[m]
        idx_arr[ks] = (sdst[m] - shalf[m] * HALF).astype(np.int16)
        lseg_arr[ks] = slseg[m].astype(np.float32)

        idx16 = np.ascontiguousarray(np.tile(idx_arr.reshape(-1, 16).T, (8, 1)))
        lsegT = np.ascontiguousarray(lseg_arr.reshape(TOT, 128).T)

        own_nodes = nodeof[k]
        c_own = counts[own_nodes]
        c_pad = np.concatenate([c_own, np.zeros(NPAD - NPC, np.int64)])
        inv = (1.0 / np.maximum(c_pad, 1)).astype(np.float32)
        inv_cnt = np.ascontiguousarray(inv.reshape(NBLK, 128).T)
        maskrow = (c_pad > 0).astype(np.float32).reshape(1, NPAD).copy()

        x_ownT = np.zeros((D, NPAD), dtype=np.float32)
        x_ownT[:, :NPC] = x[own_nodes].T

        in_maps.append(
            {
                "x_bf": x_bf,
                "x_ownT": x_ownT,
                "idx16": idx16,
                "lsegT": lsegT,
                "inv_cnt": inv_cnt,
                "maskrow": maskrow,
                "W_self_T": W_self_T,
                "W_nb_T": W_nb_T,
                "bs_row": bs_row,
                "bn_row": bn_row,
                "cmpidx": cmpidx,
                "ident": ident,
            }
        )
    return in_maps, C, CO, TOT, nodeof


def _build(C, CO, TOT):
    nc = bacc.Bacc("TRN2", target_bir_lowering=True)

    x_bf = nc.dram_tensor("x_bf", [N_NODES, D], BF16, kind="ExternalInput")
    x_ownT = nc.dram_tensor("x_ownT", [D, NPAD], F32, kind="ExternalInput")
    idx16 = nc.dram_tensor("idx16", [128, TOT * 8], I16, kind="ExternalInput")
    lsegT = nc.dram_tensor("lsegT", [128, TOT], F32, kind="ExternalInput")
    inv_cnt = nc.dram_tensor("inv_cnt", [128, NBLK], F32, kind="ExternalInput")
    maskrow = nc.dram_tensor("maskrow", [1, NPAD], F32, kind="ExternalInput")
    W_self_T = nc.dram_tensor("W_self_T", [D, D], F32, kind="ExternalInput")
    W_nb_T = nc.dram_tensor("W_nb_T", [D, D], F32, kind="ExternalInput")
    bs_row = nc.dram_tensor("bs_row", [1, D], F32, kind="ExternalInput")
    bn_row = nc.dram_tensor("bn_row", [1, D], F32, kind="ExternalInput")
    cmpidx = nc.dram_tensor("cmpidx", [128, 128], BF16, kind="ExternalInput")
    ident = nc.dram_tensor("ident", [128, 128], F32, kind="ExternalInput")
    out = nc.dram_tensor("out", [NPC, D], F32, kind="ExternalOutput")

    with tile.TileContext(nc) as tc:
        with (
            tc.tile_pool(name="consts", bufs=1) as consts,
            tc.tile_pool(name="gp", bufs=6) as gp,
            tc.tile_pool(name="ohp", bufs=6) as ohp,
            tc.tile_pool(name="mp", bufs=3) as mp,
            tc.tile_pool(name="pseg", bufs=2, space="PSUM") as pseg,
            tc.tile_pool(name="ptp", bufs=2, space="PSUM") as ptp,
            tc.tile_pool(name="pop", bufs=2, space="PSUM") as pop,
        ):
            idx_sb = consts.tile([128, TOT * 8], I16)
            nc.sync.dma_start(out=idx_sb, in_=idx16[:, :])
            lseg_sb = consts.tile([128, TOT], F32)
            nc.sync.dma_start(out=lseg_sb, in_=lsegT[:, :])
            xoT_sb = consts.tile([128, NPAD], F32)
            nc.sync.dma_start(out=xoT_sb, in_=x_ownT[:, :])
            inv_sb = consts.tile([128, NBLK], F32)
            nc.sync.dma_start(out=inv_sb, in_=inv_cnt[:, :])
            mask_sb = consts.tile([1, NPAD], F32)
            nc.sync.dma_start(out=mask_sb, in_=maskrow[:, :])
            wsT_sb = consts.tile([128, 128], F32)
            nc.sync.dma_start(out=wsT_sb, in_=W_self_T[:, :])
            wnT_sb = consts.tile([128, 128], F32)
            nc.sync.dma_start(out=wnT_sb, in_=W_nb_T[:, :])
            bs_sb = consts.tile([1, 128], F32)
            nc.sync.dma_start(out=bs_sb, in_=bs_row[:, :])
            bn_sb = consts.tile([1, 128], F32)
            nc.sync.dma_start(out=bn_sb, in_=bn_row[:, :])
            cmp_sb = consts.tile([128, 128], BF16)
            nc.sync.dma_start(out=cmp_sb, in_=cmpidx[:, :])
            id_sb = consts.tile([128, 128], F32)
            nc.sync.dma_start(out=id_sb, in_=ident[:, :])
            ones_sb = consts.tile([1, 128], F32)
            nc.vector.memset(ones_sb, 1.0)

            for b in range(NBLK):
                nmm = int(C[b][0] + C[b][1])
                mmi = 0
                if nmm > 0:
                    seg = pseg.tile([128, 128], F32, tag="seg")
                for h in (0, 1):
                    cb = int(C[b][h])
                    if cb == 0:
                        continue
                    co = int(CO[b][h])
                    g = gp.tile([128, cb, 128], BF16, tag="g")
                    nc.gpsimd.dma_gather(
                        out_ap=g[:, :, :],
                        in_ap=x_bf[h * HALF : (h + 1) * HALF, :],
                        idxs_ap=idx_sb[:, co * 8 : (co + cb) * 8],
                        num_idxs=cb * 128,
                        num_idxs_reg=cb * 128,
                        elem_size=D,
                        single_packet=False,
                    )
                    for c in range(cb):
                        oh = ohp.tile([128, 128], BF16, tag="oh")
                        nc.vector.tensor_scalar(
                            out=oh,
                            in0=cmp_sb,
                            scalar1=lseg_sb[:, co + c : co + c + 1],
                            scalar2=None,
                            op0=mybir.AluOpType.is_equal,
                        )
                        nc.tensor.matmul(
                            seg,
                            lhsT=oh,
                            rhs=g[:, c, :],
                            start=(mmi == 0),
                            stop=(mmi == nmm - 1),
                        )
                        mmi += 1

                if nmm > 0:
                    mean = mp.tile([128, 128], F32, tag="mean")
                    nc.vector.tensor_scalar_mul(mean, seg[:, :], inv_sb[:, b : b + 1])
                    pt = ptp.tile([128, 128], F32, tag="pt")
                    nc.tensor.transpose(pt, mean, id_sb)
                    meanT = mp.tile([128, 128], F32, tag="meanT")
                    nc.vector.tensor_copy(meanT, pt[:, :])

                po = pop.tile([128, 128], F32, tag="po")
                nc.tensor.matmul(
                    po, lhsT=xoT_sb[:, b * 128 : (b + 1) * 128], rhs=wsT_sb,
                    start=True, stop=False,
                )
                nc.tensor.matmul(po, lhsT=ones_sb, rhs=bs_sb, start=False, stop=False)
                if nmm > 0:
                    nc.tensor.matmul(po, lhsT=meanT, rhs=wnT_sb, start=False, stop=False)
                nc.tensor.matmul(
                    po, lhsT=mask_sb[:, b * 128 : (b + 1) * 128], rhs=bn_sb,
                    start=False, stop=True,
                )

                ob = mp.tile([128, 128], F32, tag="ob")
                nc.scalar.activation(ob, po[:, :], mybir.ActivationFunctionType.Relu)
                nrows = min(128, NPC - b * 128)
                nc.sync.dma_start(
                    out=out[b * 128 : b * 128 + nrows, :], in_=ob[:nrows, :]
                )

    nc.finalize()
    return nc


def _assemble(results, nodeof):
    full = np.empty((N_NODES, D), dtype=np.float32)
    for k in range(N_CORES):
        full[nodeof[k]] = results[k]["out"]
    return full


def kernel(x, edge_index, W_self, b_self, W_neighbor, b_neighbor):
    in_maps, C, CO, TOT, nodeof = _preprocess(
        x, edge_index, W_self, b_self, W_neighbor, b_neighbor
    )
    nc = _build(C, CO, TOT)
    res = run_bass_kernel_spmd(nc, in_maps, core_ids=list(range(N_CORES)))
    return _assemble(res.results, nodeof)


# exposed for test.py so the perf harness can reuse the prepared pieces
def _prepare(x, edge_index, W_self, b_self, W_neighbor, b_neighbor):
    in_maps, C, CO, TOT, nodeof = _preprocess(
        x, edge_index, W_self, b_self, W_neighbor, b_neighbor
    )
    nc = _build(C, CO, TOT)
    return nc, in_maps, nodeof
